# revision 14
# baseline (speedup 1.0000x reference)
"""CUBA-LIF recurrent spiking network kernel for Trainium2 (8 NeuronCores).

Problem: x:[32,512,1024] f32, W:[512,512], R:[512,512]
  z = einsum('nct,oc->tno', x, W)
  scan over t: rec = s @ R.T; i = .75 i + z_t + rec;
               v = .9 v (1-s) + i; s = (v >= 1)
  out[n,o,t] = s_{t-1}[n,o]  (zero at t=0)

Strategy: data-parallel over batch N across 8 cores (4 per core).
All on-chip state is in [128 partitions = C_out%128, (chunk, n)] layout so
the recurrent matmul (R stationary, s^T moving) needs no transposes and the
per-step elementwise chain is tiny. R is split into three bf16 matrices
summing exactly to R (spikes are binary, so every product is exact and the
arithmetic stays fp32-faithful) — bf16 weight loads run ~15x faster than
fp32 ones on the PE. The only on-critical-path op per step is a single
fused DVE compare: s = ((rec - 1) >= -(w + zi)); everything else (i, v, w,
m, zi) is computed off-path on DVE/GPSIMD/ACT during the next step's
matmuls. Spikes are written as bf16 directly into quarter buffers,
converted+DMA'd out (delay-shifted) as each quarter completes.
"""

import numpy as np

N_FULL, C_IN, C_OUT, T_FULL = 32, 512, 512, 1024
N_CORES = 8
N_LOC = N_FULL // N_CORES  # 4
P = 128
NCH = C_OUT // P  # 4
KCH = C_IN // P   # 4


def build_program(T=T_FULL, tz=256, tq=256, rec_mode="bf16x3"):
    """Build the per-core Bass/Tile program (SPMD: same program, sharded data)."""
    import concourse.bacc as bacc
    import concourse.tile as tile
    import concourse.mybir as mybir

    f32 = mybir.dt.float32
    bf16 = mybir.dt.bfloat16
    Alu = mybir.AluOpType
    Act = mybir.ActivationFunctionType

    nc = bacc.Bacc("TRN2", target_bir_lowering=False, debug=False)

    xs = nc.dram_tensor("xs", [N_LOC, C_IN, T], f32, kind="ExternalInput").ap()
    WT = nc.dram_tensor("WT", [C_IN, C_OUT], f32, kind="ExternalInput").ap()
    out_d = nc.dram_tensor("out", [N_LOC, C_OUT, T], f32, kind="ExternalOutput").ap()
    n_pass = {"bf16x2": 2, "bf16x3": 3}[rec_mode]
    rt_srcs = [
        nc.dram_tensor(f"RT_p{pi}", [C_OUT, C_OUT], bf16, kind="ExternalInput").ap()
        for pi in range(n_pass)
    ]

    n_tb = T // tz           # z-compute time blocks
    n_steps = T - 1          # s_{T-1} is discarded by the delay shift
    n_q = (n_steps + tq - 1) // tq   # spike-buffer quarters

    with tile.TileContext(nc) as tc:
        with (
            tc.tile_pool(name="weights", bufs=1) as wpool,
            tc.tile_pool(name="bigbuf", bufs=1) as bigpool,
            tc.tile_pool(name="xin", bufs=2) as xpool,
            tc.tile_pool(name="state", bufs=3) as spool,
            tc.tile_pool(name="stg", bufs=1) as stgpool,
            tc.tile_pool(name="zpsum", bufs=2, space="PSUM") as zpsum,
            tc.tile_pool(name="recpsum", bufs=3, space="PSUM") as rpsum,
        ):
            # ---- resident weights ----
            wt_sb = []
            for kc in range(KCH):
                w = wpool.tile([P, C_OUT], f32, tag=f"wt{kc}", name=f"wt{kc}")
                nc.sync.dma_start(w[:], WT[kc * P:(kc + 1) * P, :])
                wt_sb.append(w)
            rt_sb = []
            for pi, src in enumerate(rt_srcs):
                tiles = []
                for oc in range(NCH):
                    r = wpool.tile([P, C_OUT], bf16, tag=f"rt{pi}_{oc}",
                                   name=f"rt{pi}_{oc}")
                    nc.sync.dma_start(r[:], src[oc * P:(oc + 1) * P, :])
                    tiles.append(r)
                rt_sb.append(tiles)
            bias09 = wpool.tile([P, 1], f32, tag="bias09", name="bias09")
            nc.vector.memset(bias09[:], 0.9)

            # ---- big SBUF buffers ----
            z_buf = bigpool.tile([P, T, NCH, N_LOC], f32, tag="zbuf", name="zbuf")
            # spike history, bf16, quartered: sq[q][:, trel, cc, n] = s_{q*tq+trel}
            sq = [bigpool.tile([P, min(tq, n_steps - q * tq), NCH, N_LOC], bf16,
                               tag=f"sq{q}", name=f"sq{q}")
                  for q in range(n_q)]

            # ---- phase 1: z = x @ W^T, transposed into z_buf ----
            for tb in range(n_tb):
                x_sb = []
                for kc in range(KCH):
                    xt = xpool.tile([P, N_LOC, tz], f32, tag=f"x{kc}",
                                    name=f"x_{tb}_{kc}")
                    nc.sync.dma_start(
                        xt[:],
                        xs[:, kc * P:(kc + 1) * P, tb * tz:(tb + 1) * tz]
                        .rearrange("n p t -> p n t"),
                    )
                    x_sb.append(xt)
                for cc in range(NCH):
                    for n in range(N_LOC):
                        pz = zpsum.tile([P, tz], f32, tag="zp",
                                        name=f"zp_{tb}_{cc}_{n}")
                        for kc in range(KCH):
                            nc.tensor.matmul(
                                pz[:],
                                wt_sb[kc][:, cc * P:(cc + 1) * P],
                                x_sb[kc][:, n, :],
                                start=(kc == 0), stop=(kc == KCH - 1),
                            )
                        nc.scalar.copy(
                            out=z_buf[:, tb * tz:(tb + 1) * tz, cc, n],
                            in_=pz[:],
                        )

            # ---- phase 2: the scan ----
            # state slots: 0=i, 1=v, 2=hneg(=-(w+zi)), 3=m, 4=w, 5=zi
            st_prev = spool.tile([P, 6, NCH, N_LOC], f32, tag="st", name="st_init")
            sT_init = spool.tile([P, NCH, N_LOC], bf16, tag="sTi", name="sT_init")
            nc.vector.memset(sT_init[:], 0.0)                  # s_{-1} = 0
            nc.vector.memset(st_prev[:, 4], 0.0)               # w_0 = 0
            nc.vector.tensor_copy(st_prev[:, 5], z_buf[:, 0])  # zi_0 = z_0
            nc.vector.tensor_scalar(st_prev[:, 2], z_buf[:, 0],
                                    -1.0, None, Alu.mult)      # hneg_0 = -z_0
            zero16 = spool.tile([P, NCH, N_LOC], f32, tag="z16", name="zero16")
            nc.vector.memset(zero16[:], 0.0)
            for cc in range(NCH):                              # out[..., 0] = 0
                nc.sync.dma_start(
                    out_d[:, cc * P:(cc + 1) * P, 0].rearrange("n p -> p n"),
                    zero16[:, cc])

            for t in range(n_steps):
                q, trel = t // tq, t % tq
                pr = rpsum.tile([P, NCH, N_LOC], f32, tag="rp", name=f"rp_{t}")
                rhs = sT_init[:] if t == 0 else \
                    sq[(t - 1) // tq][:, (t - 1) % tq]
                for cc in range(NCH):
                    k = 0
                    for rtiles in rt_sb:
                        for oc in range(NCH):
                            nc.tensor.matmul(
                                pr[:, cc, :],
                                rtiles[oc][:, cc * P:(cc + 1) * P],
                                rhs[:, oc, :],
                                start=(k == 0), stop=(k == n_pass * NCH - 1),
                            )
                            k += 1

                st = spool.tile([P, 6, NCH, N_LOC], f32, tag="st", name=f"st_{t}")
                # ON-PATH: s_t = ((rec - 1) >= -(w + zi))  [1 DVE op]
                nc.vector.scalar_tensor_tensor(
                    sq[q][:, trel], pr[:], 1.0, st_prev[:, 2],
                    Alu.subtract, Alu.is_ge)
                # off-path
                nc.vector.tensor_add(st[:, 0], pr[:], st_prev[:, 5])  # i
                if t < n_steps - 1:
                    nc.vector.scalar_tensor_tensor(
                        st[:, 5], st[:, 0], 0.75, z_buf[:, t + 1],
                        Alu.mult, Alu.add)                            # zi'
                    nc.scalar.activation(
                        st[:, 3], sq[q][:, trel], Act.Relu,
                        bias=bias09[:], scale=-0.9)                   # m = .9(1-s)
                    nc.gpsimd.tensor_add(st[:, 1], st_prev[:, 4], st[:, 0])  # v = w + i
                    nc.gpsimd.tensor_mul(st[:, 4], st[:, 1], st[:, 3])    # w'
                    nc.vector.scalar_tensor_tensor(
                        st[:, 2], st[:, 4], -1.0, st[:, 5],
                        Alu.mult, Alu.subtract)                       # hneg'
                st_prev = st

                # quarter finished -> convert bf16->f32 and stream out
                if trel == sq[q].shape[1] - 1:
                    cnt = sq[q].shape[1]
                    stg = stgpool.tile([P, NCH, N_LOC, tq], f32, tag="stg",
                                       name=f"stg_{q}")
                    nc.scalar.copy(
                        out=stg[:, :, :, 0:cnt],
                        in_=sq[q][:].rearrange("p t c n -> p c n t"))
                    for cc in range(NCH):
                        nc.sync.dma_start(
                            out_d[:, cc * P:(cc + 1) * P,
                                  q * tq + 1:q * tq + 1 + cnt]
                            .rearrange("n p t -> p n t"),
                            stg[:, cc, :, 0:cnt])

    nc.compile()
    return nc


_NC_CACHE = {}


def _get_program(T=T_FULL, rec_mode="bf16x3"):
    key = (T, rec_mode)
    if key not in _NC_CACHE:
        tz = min(256, T)
        tq = min(256, T)
        _NC_CACHE[key] = build_program(T=T, tz=tz, tq=tq, rec_mode=rec_mode)
    return _NC_CACHE[key]


def _split_bf16(a, n_pass=3):
    """Split fp32 matrix into n_pass bf16 matrices summing (near-)exactly to a."""
    import ml_dtypes
    parts = []
    rem = a.astype(np.float32)
    for _ in range(n_pass):
        p = rem.astype(ml_dtypes.bfloat16)
        parts.append(p)
        rem = (rem - p.astype(np.float32)).astype(np.float32)
    return parts


def _run(x, W, R, T=T_FULL, rec_mode="bf16x3", trace=False):
    from concourse.bass_utils import run_bass_kernel_spmd

    x = np.ascontiguousarray(np.asarray(x, dtype=np.float32))
    W = np.asarray(W, dtype=np.float32)
    R = np.asarray(R, dtype=np.float32)
    WT = np.ascontiguousarray(W.T)
    RT = np.ascontiguousarray(R.T)

    nc = _get_program(T=T, rec_mode=rec_mode)
    base = {"WT": WT}
    for pi, part in enumerate(_split_bf16(RT, int(rec_mode[-1]))):
        base[f"RT_p{pi}"] = part
    in_maps = [
        {**base, "xs": np.ascontiguousarray(x[k * N_LOC:(k + 1) * N_LOC])}
        for k in range(N_CORES)
    ]
    res = run_bass_kernel_spmd(nc, in_maps, list(range(N_CORES)), trace=trace)
    out = np.concatenate([res.results[k]["out"] for k in range(N_CORES)], axis=0)
    return out.astype(np.float32), res


def kernel(x, W, R):
    out, _ = _run(x, W, R)
    return out
